# revision 1
# baseline (speedup 1.0000x reference)
"""Fused AttnBlock kernel for Trainium2, SPMD over 8 NeuronCores.

Problem: x[4,512,64,64] -> GroupNorm(32) -> q,k,v 1x1 convs -> attention
over HW=4096 tokens -> out proj -> residual.  ~172 GFLOP total.

Sharding: core c handles batch b=c//2 and query-half h=c%2.  The host
rolls the spatial axis by 2048*h so every core runs the identical
program on "queries = columns 0..2047"; softmax/attention are
permutation-invariant over keys, so rolled keys give identical results.

Device algorithm (per core, everything fused on-chip).  The q/k and
v/o projections are folded algebraically (host):
  G = Wk^T Wq, gb = Wk^T bq;  Wvo = Wo Wv, bo2 = Wo bv + bo.
GroupNorm is affine per channel (h = A.x + B), so the key-side and
v-side normalizes fold away entirely (raw-x attention):
  scoresT = h^T(G h_q + gb) = x^T[A.(G h_q + gb)] + const(query)
    -> the per-query constant cancels in softmax and is DROPPED;
       A and A.gb fold into the m-projection's PSUM-drain activation.
  u = V attnT with V = A.x + B  ->  u = A.(x attnT) + B.usum
    -> the u-matmul consumes RAW xT; A folds into the h2 drain and
       B becomes a Wvo.B correction in the final bias (16 one-column
       matmuls, off the critical path).
Only the 2048 query columns of h are normalized (feeds the G-matmul).

Phases:
  A. x load in [P,1024] chunks (cb3 first per chunk group); cb0-2
     stats via DVE bn_stats per chunk, cb3 via ACT Identity/Square
     accum_out (reduced on gpsimd, in parallel with the DVE stats);
     group reduce/broadcast via tiny indicator matmuls on the PE,
     accumulated as soon as each channel block's stats land.
  B. m = A.(G h_q + gb) (64 matmuls, ACT drain applies scale+bias);
     then Wvo.B bias-correction matmuls.
  C. Flash-style attention over 4 query blocks of 512, depth-6
     software pipeline: scoresT = x^T m in PSUM -> exp via ACT -> eT
     bf16 (split buffers); u = xT^T eT; usum via [P,P]-ones matmuls
     interleaved into the pipeline (last 6 kept as a drain to cover
     the h2-copy latency); 1/usum commutes through the out-proj and
     is applied in the final DVE op together with bias + residual.
     Out-proj PSUM reuses the (drained) u accumulator banks so the
     four out-proj groups never serialize on a single bank.
"""

import os
import numpy as np

import concourse.bass as bass
import concourse.tile as tile
from concourse import bacc, mybir
from concourse.bass_utils import run_bass_kernel_spmd

F32 = mybir.dt.float32
BF16 = mybir.dt.bfloat16
F16 = mybir.dt.float16
AF = mybir.ActivationFunctionType
OP = mybir.AluOpType

C = 512          # channels
HW = 4096        # tokens
NG = 32          # groups
GS = 16          # channels per group
EPS = 1e-5
P = 128          # partitions
NCB = C // P     # channel blocks = 4
IQ = HW // 2     # queries per core = 2048
NIB = IQ // 512  # query blocks of 512 = 4
NJB = HW // P    # key blocks of 128 = 32
FD = 512         # matmul free dim / PSUM bank
SCALE = float(C) ** -0.5
SD = 6           # software pipeline depth (exp -> u lag, in key blocks)
USUM_TAIL = 6    # usum matmuls kept as a post-pipeline drain

LAST_EXEC_TIME_NS = None
LAST_RESULTS = None
_NC_CACHE = None


def _emit(tc):
    nc = tc.nc
    xd = nc.dram_tensor("x", [C, HW], F32, kind="ExternalInput")
    xhd = nc.dram_tensor("xh", [C, HW], BF16, kind="ExternalInput")
    xhTd = nc.dram_tensor("xhT", [HW, C], BF16, kind="ExternalInput")
    wgd = nc.dram_tensor("gT", [C, C], F16, kind="ExternalInput")
    wvod = nc.dram_tensor("wvoT", [C, C], BF16, kind="ExternalInput")
    vecsd = nc.dram_tensor("vecs", [P, NCB * 5], F32, kind="ExternalInput")
    indrd = nc.dram_tensor("indr", [P, NCB * NG], F32, kind="ExternalInput")
    indbd = nc.dram_tensor("indb", [NG, C], F32, kind="ExternalInput")
    yd = nc.dram_tensor("y", [C, IQ], F32, kind="ExternalOutput")

    with (
        tc.tile_pool(name="const", bufs=1) as constp,
        tc.tile_pool(name="wpool", bufs=1) as wpool,
        tc.tile_pool(name="projp", bufs=1) as projp,
    ):
        # ---- constants ----
        eps_sb = constp.tile([NG, 1], F32, name="eps_sb")
        nc.vector.memset(eps_sb, EPS)
        # dummy abs_reciprocal_sqrt: loads the table set that serves the
        # stats Identity/Square AND the group-norm rsig below -- no further
        # ACT table load until the (hidden) exp warm at the end of phase B
        warm_sb = constp.tile([1, 1], F32, name="warm_sb")
        nc.scalar.activation(warm_sb, eps_sb[0:1, 0:1], AF.Abs_reciprocal_sqrt,
                             bias=0.0, scale=1.0)
        # [P, P] of ones: usum matmul yields 128 identical rows -- the
        # reciprocal then IS the partition broadcast
        ones_bf = constp.tile([P, P], BF16, name="ones_bf")
        nc.vector.memset(ones_bf, 1.0)
        vecs_sb = constp.tile([P, NCB, 5], F32, name="vecs_sb")
        nc.gpsimd.dma_start(vecs_sb, vecsd.rearrange("p (cb f) -> p cb f", f=5))
        indr_sb = constp.tile([P, NCB * NG], F32, name="indr_sb")
        nc.gpsimd.dma_start(indr_sb, indrd[:, :])
        indb_sb = constp.tile([NG, C], F32, name="indb_sb")
        nc.gpsimd.dma_start(indb_sb, indbd[:, :])

        def gb_ap(cb):
            return vecs_sb[:, cb, 0:1]

        def bo2_ap(cb):
            return vecs_sb[:, cb, 2:3]

        def gnw_ap(cb):
            return vecs_sb[:, cb, 3:4]

        def gnb_ap(cb):
            return vecs_sb[:, cb, 4:5]

        # persistent per-channel vectors consumed in phase C
        A_sb = constp.tile([P, NCB], F32, name="A_sb")
        bgA_sb = constp.tile([P, NCB], F32, name="bgA_sb")   # A*gb
        Bb16 = constp.tile([P, NCB], BF16, name="Bb16")
        bo2p = constp.tile([P, NCB], F32, name="bo2p")       # bo2 + Wvo B

        def A_ap(cb):
            return A_sb[:, cb:cb + 1]

        # ---- persistent weight tiles ----
        w_bf = {}
        for wname, wd_, wdt in (("g", wgd, F16), ("vo", wvod, BF16)):
            w_bf[wname] = []
            for cb in range(NCB):
                t = wpool.tile([P, C], wdt, tag=f"w{wname}{cb}", name=f"w{wname}{cb}")
                w_bf[wname].append(t)

        # ---- persistent tiles: m (A-folded), h (queries only), raw x both ways
        m_bf = [projp.tile([P, IQ], BF16, tag=f"m{cb}", name=f"m{cb}") for cb in range(NCB)]
        h_bf = [projp.tile([P, IQ], F16, tag=f"h{cb}", name=f"h{cb}") for cb in range(NCB)]
        xs = [projp.tile([P, HW], BF16, tag=f"x{cb}", name=f"x{cb}") for cb in range(NCB)]
        xt = [projp.tile([P, 8, FD], BF16, tag=f"xt{g}", name=f"xt{g}") for g in range(NCB)]

        # =========== phase A+B scope ===========
        with (
            tc.tile_pool(name="statp", bufs=1) as statp,
            tc.tile_pool(name="psB", bufs=6, space="PSUM") as psB,
        ):
            # ---- A: x load chunked; stats streamed per chunk ----
            # DVE bn_stats: cb0-2 (24 ops of 512).  ACT Identity/Square
            # accum: cb3 per [P,2048] half (raw sums, reduced on gpsimd;
            # host scales that indicator block by 1/(GS*HW) so the group
            # reduce consumes raw totals).
            bsts = [statp.tile([P, 8, 6], F32, tag=f"bst{cb}", name=f"bst{cb}")
                    for cb in range(NCB - 1)]
            park = statp.tile([P, 2048], BF16, name="park")
            accs = {3: []}
            for s2 in range(4):
                sl2 = slice(s2 * 1024, (s2 + 1) * 1024)
                for cb in range(NCB):
                    nc.sync.dma_start(xs[cb][:, sl2], xhd[cb * P:(cb + 1) * P, sl2])
                    if cb == 3:
                        continue
                    for half in range(2):
                        s = 2 * s2 + half
                        sl = slice(s * 512, (s + 1) * 512)
                        nc.vector.bn_stats(bsts[cb][:, s, :], xs[cb][:, sl])
                    if s2 == 3:
                        # aggregate as soon as this block's stats land
                        mv = statp.tile([P, 2], F32, tag=f"mv{cb}",
                                        name=f"mv{cb}")
                        nc.vector.bn_aggr(mv, bsts[cb])
                        nc.vector.scalar_tensor_tensor(
                            mv[:, 1:2], mv[:, 0:1], mv[:, 0:1], mv[:, 1:2],
                            op0=OP.mult, op1=OP.add)
                        accs[cb] = mv
            for half in range(2):
                sl = slice(half * 2048, (half + 1) * 2048)
                a_s = statp.tile([P, 1], F32, name=f"accs{half}")
                a_q = statp.tile([P, 1], F32, name=f"accq{half}")
                nc.scalar.activation(park, xs[3][:, sl], AF.Identity,
                                     bias=0.0, scale=1.0, accum_out=a_s)
                nc.scalar.activation(park, xs[3][:, sl], AF.Square,
                                     bias=0.0, scale=1.0, accum_out=a_q)
                accs[3].append((a_s, a_q))

            # weight + xhT load AFTER the x chunks on the SAME (in-order
            # sync) queue: phase A is HBM-BW bound and these are needed later
            for wname, wd_ in (("g", wgd), ("vo", wvod)):
                for cb in range(NCB):
                    nc.sync.dma_start(w_bf[wname][cb], wd_[cb * P:(cb + 1) * P, :])
            for g in range(NCB):
                nc.sync.dma_start(
                    xt[g],
                    xhTd[g * 1024:(g + 1) * 1024, :].rearrange(
                        "(sub p) c -> p sub c", p=P))

            # cb3 raw-sum reduction on gpsimd, parallel to DVE bn_stats
            st3 = statp.tile([P, 2], F32, name="st3")
            nc.gpsimd.tensor_add(st3[:, 0:1], accs[3][0][0], accs[3][1][0])
            nc.gpsimd.tensor_add(st3[:, 1:2], accs[3][0][1], accs[3][1][1])

            # group-stat reduce: accumulate each channel block as its
            # aggregate lands (cb0 first, cb2 last; cb3 from the ACT path)
            gst_ps = psB.tile([NG, 2], F32, tag="pp", name="gst_ps")
            nc.tensor.matmul(gst_ps, indr_sb[:, 0 * NG:1 * NG], accs[0],
                             start=True, stop=False)
            nc.tensor.matmul(gst_ps, indr_sb[:, 3 * NG:4 * NG], st3,
                             start=False, stop=False)
            nc.tensor.matmul(gst_ps, indr_sb[:, 1 * NG:2 * NG], accs[1],
                             start=False, stop=False)
            nc.tensor.matmul(gst_ps, indr_sb[:, 2 * NG:3 * NG], accs[2],
                             start=False, stop=True)
            # group post-processing: mu, rsig
            gst = statp.tile([NG, 2], F32, name="gst")
            nc.vector.tensor_copy(gst, gst_ps)
            mumu = statp.tile([NG, 1], F32, name="mumu")
            nc.vector.tensor_mul(mumu, gst[:, 0:1], gst[:, 0:1])
            varg = statp.tile([NG, 1], F32, name="varg")
            nc.vector.tensor_sub(varg, gst[:, 1:2], mumu)
            # rsig = 1/sqrt(var+eps) in ONE table-resident ACT op (no Sqrt
            # table load mid-chain, no DVE reciprocal)
            grhs = statp.tile([NG, 2], F32, name="grhs")
            nc.vector.tensor_copy(grhs[:, 0:1], gst[:, 0:1])
            nc.scalar.activation(grhs[:, 1:2], varg, AF.Abs_reciprocal_sqrt,
                                 bias=eps_sb, scale=1.0)

            B_ts = []
            for cb in range(NCB):
                ms_ps = psB.tile([P, 2], F32, tag="pp", name=f"msps{cb}")
                nc.tensor.matmul(ms_ps, indb_sb[:, cb * P:(cb + 1) * P], grhs,
                                 start=True, stop=True)
                A_t = A_ap(cb)
                nc.vector.tensor_mul(A_t, ms_ps[:, 1:2], gnw_ap(cb))
                B_t = statp.tile([P, 1], F32, tag=f"B{cb}", name=f"B{cb}")
                nc.vector.tensor_mul(B_t, ms_ps[:, 0:1], A_t)
                nc.vector.tensor_sub(B_t, gnb_ap(cb), B_t)
                nc.vector.tensor_mul(bgA_sb[:, cb:cb + 1], A_t, gb_ap(cb))
                nc.gpsimd.tensor_copy(Bb16[:, cb:cb + 1], B_t)
                B_ts.append(B_t)

            # normalize h for the query columns only (s-major so the m-proj
            # can start on early blocks); mostly DVE (ACT still has stats)
            for s in range(NIB):
                for cb in range(NCB):
                    sl = slice(s * FD, (s + 1) * FD)
                    if (s * NCB + cb) % 4 == 3:
                        nc.scalar.activation(h_bf[cb][:, sl], xs[cb][:, sl],
                                             AF.Identity, bias=B_ts[cb],
                                             scale=A_ap(cb))
                    else:
                        nc.vector.tensor_scalar(h_bf[cb][:, sl], xs[cb][:, sl],
                                                A_ap(cb), B_ts[cb],
                                                op0=OP.mult, op1=OP.add)

            # ---- B: m = A.(G h_q + gb), one fused projection (ib-major so
            # the scores' first query block drains first; drains split
            # ACT/DVE so ACT is clear when the exp pipeline starts) ----
            for ib in range(NIB):
                for cb in range(NCB):
                    ps = psB.tile([P, FD], F32, tag="pp", name=f"mps{cb}_{ib}")
                    for cpb in range(NCB):
                        nc.tensor.matmul(ps, w_bf["g"][cpb][:, cb * P:(cb + 1) * P],
                                         h_bf[cpb][:, ib * FD:(ib + 1) * FD],
                                         start=(cpb == 0), stop=(cpb == NCB - 1))
                    mo = m_bf[cb][:, ib * FD:(ib + 1) * FD]
                    if ib < 2:
                        nc.scalar.activation(mo, ps, AF.Identity,
                                             bias=bgA_sb[:, cb:cb + 1],
                                             scale=A_ap(cb))
                    else:
                        nc.vector.tensor_scalar(mo, ps, A_ap(cb),
                                                bgA_sb[:, cb:cb + 1],
                                                op0=OP.mult, op1=OP.add)

            # Wvo.B bias correction (u-path's +B term commuted through the
            # out-proj); tiny one-column matmuls, off the critical path
            for cob in range(NCB):
                wb_ps = psB.tile([P, 2], F32, tag="pp", name=f"wbps{cob}")
                for ob in range(NCB):
                    nc.tensor.matmul(wb_ps[:, 0:1],
                                     w_bf["vo"][ob][:, cob * P:(cob + 1) * P],
                                     Bb16[:, ob:ob + 1],
                                     start=(ob == 0), stop=(ob == NCB - 1))
                nc.vector.tensor_add(bo2p[:, cob:cob + 1], wb_ps[:, 0:1],
                                     bo2_ap(cob))


        # =========== phase C scope ===========
        with (
            tc.tile_pool(name="pscp", bufs=3, space="PSUM") as pscp,
            tc.tile_pool(name="psup", bufs=1, space="PSUM") as psup,
            tc.tile_pool(name="epool", bufs=1) as epool,
            tc.tile_pool(name="cpool", bufs=1) as cpool,
        ):
            for ib in range(NIB):
                # two half-buffers: the second half of this block's exps can
                # overlap the first half of the next block's scores
                eTa = epool.tile([P, NJB // 2, FD], BF16, tag="eTa", name=f"eTa{ib}")
                eTb = epool.tile([P, NJB // 2, FD], BF16, tag="eTb", name=f"eTb{ib}")

                def eT_sl(jb):
                    return (eTa if jb < NJB // 2 else eTb)[:, jb % (NJB // 2), :]

                us = [psup.tile([P, FD], F32, tag=f"u{ob}", name=f"u{ib}_{ob}")
                      for ob in range(NCB)]
                usum = psup.tile([P, FD], F32, tag="usum", name=f"usum{ib}")
                NDRAIN = NJB - USUM_TAIL
                # software pipeline: u-matmuls consume exps from SD j-blocks
                # ago; usum matmuls ride along (except the last USUM_TAIL,
                # kept as a drain that covers the h2-copy latency)
                for step in range(NJB + SD):
                    if step < NJB:
                        jb = step
                        sps = pscp.tile([P, FD], F32, tag="sc", name=f"s{ib}_{jb}")
                        for cb in range(NCB):
                            nc.tensor.matmul(sps, xs[cb][:, jb * P:(jb + 1) * P],
                                             m_bf[cb][:, ib * FD:(ib + 1) * FD],
                                             start=(cb == 0), stop=(cb == NCB - 1))
                        nc.scalar.activation(eT_sl(jb), sps, AF.Exp, scale=SCALE)
                    if step >= SD:
                        jb2 = step - SD
                        for cb in range(NCB):
                            nc.tensor.matmul(us[cb],
                                             xt[jb2 // 8][:, jb2 % 8, cb * P:(cb + 1) * P],
                                             eT_sl(jb2),
                                             start=(jb2 == 0), stop=(jb2 == NJB - 1))
                        if jb2 < NDRAIN:
                            nc.tensor.matmul(usum, ones_bf, eT_sl(jb2),
                                             start=(jb2 == 0), stop=False)
                for jb2 in range(NDRAIN, NJB):
                    nc.tensor.matmul(usum, ones_bf, eT_sl(jb2),
                                     start=False, stop=(jb2 == NJB - 1))
                # h2 = A.u (unnormalized; 1/usum commutes through the out-proj)
                h2 = []
                for ob in range(NCB):
                    t = cpool.tile([P, FD], BF16, tag=f"h2_{ob}", bufs=2,
                                   name=f"h2_{ib}_{ob}")
                    if ob % 2 == 0:
                        nc.scalar.activation(t, us[ob], AF.Identity, bias=0.0,
                                             scale=A_ap(ob))
                    else:
                        nc.vector.tensor_scalar(t, us[ob], A_ap(ob), 0.0,
                                                op0=OP.mult, op1=OP.add)
                    h2.append(t)
                rb_sb = cpool.tile([P, FD], F32, tag="rb_sb", bufs=2, name=f"rbsb{ib}")
                rscr = cpool.tile([P, FD], F32, tag="rscr", bufs=2, name=f"rscr{ib}")
                nc.vector.reciprocal_approx_accurate(rb_sb, usum, rscr)
                # out-proj (unnormalized) then scale + bias + residual; the
                # out-proj PSUM reuses the drained u banks so the four groups
                # never serialize on one bank
                for cob in range(NCB):
                    ops = psup.tile([P, FD], F32, tag=f"u{cob}", name=f"o{ib}_{cob}")
                    for ob in range(NCB):
                        nc.tensor.matmul(ops, w_bf["vo"][ob][:, cob * P:(cob + 1) * P],
                                         h2[ob], start=(ob == 0), stop=(ob == NCB - 1))
                    xres = cpool.tile([P, FD], F32, tag="xres", bufs=4, name=f"xres{ib}_{cob}")
                    nc.sync.dma_start(xres, xd[cob * P:(cob + 1) * P, ib * FD:(ib + 1) * FD])
                    scaled = cpool.tile([P, FD], F32, tag="scaled", bufs=4, name=f"sc{ib}_{cob}")
                    outt = cpool.tile([P, FD], F32, tag="outt", bufs=4, name=f"outt{ib}_{cob}")
                    # last block: half-granularity so the final DVE ops and
                    # output DMAs pipeline instead of serializing the tail
                    nh = 2 if ib == NIB - 1 else 1
                    hw_ = FD // nh
                    for hh in range(nh):
                        hsl = slice(hh * hw_, (hh + 1) * hw_)
                        nc.vector.tensor_mul(scaled[:, hsl], ops[:, hsl], rb_sb[:, hsl])
                        nc.vector.scalar_tensor_tensor(outt[:, hsl], scaled[:, hsl],
                                                       bo2p[:, cob:cob + 1],
                                                       xres[:, hsl],
                                                       op0=OP.add, op1=OP.add)
                        nc.sync.dma_start(
                            yd[cob * P:(cob + 1) * P,
                               ib * FD + hh * hw_:ib * FD + (hh + 1) * hw_],
                            outt[:, hsl])


def _build_nc():
    global _NC_CACHE
    if _NC_CACHE is not None:
        return _NC_CACHE
    nc = bacc.Bacc("TRN2", target_bir_lowering=False, num_devices=8)
    with tile.TileContext(nc) as tc:
        _emit(tc)
    nc.compile()
    _NC_CACHE = nc
    return nc


def _host_inputs(x, gn_w, gn_b, wq, bq, wk, bk, wv, bv, wo, bo):
    """Build the per-core input maps (host-side layout prep only)."""
    B = x.shape[0]
    xs = np.ascontiguousarray(np.asarray(x, dtype=np.float32).reshape(B, C, HW))

    import ml_dtypes

    def t16(a):
        return np.ascontiguousarray(
            np.asarray(a, dtype=np.float32).T.astype(ml_dtypes.bfloat16))

    wq64 = np.asarray(wq, np.float64)
    wk64 = np.asarray(wk, np.float64)
    # gT = (Wk^T Wq)^T = Wq^T Wk: the q and k projections fused into one;
    # gb = Wk^T bq reproduces the per-key bias term (bk cancels in softmax)
    gT = np.ascontiguousarray(wq64.T @ wk64).astype(np.float16)
    gb = (wk64.T @ np.asarray(bq, np.float64)).astype(np.float32)
    wvoT = t16(np.asarray(wo, np.float64) @ np.asarray(wv, np.float64))
    bo2 = (np.asarray(wo, dtype=np.float64) @ np.asarray(bv, dtype=np.float64)
           + np.asarray(bo, dtype=np.float64)).astype(np.float32)

    vecs = np.zeros((P, NCB, 5), np.float32)
    for cb in range(NCB):
        sl = slice(cb * P, (cb + 1) * P)
        vecs[:, cb, 0] = gb[sl]
        vecs[:, cb, 1] = np.asarray(bk, np.float32)[sl]
        vecs[:, cb, 2] = bo2[sl]
        vecs[:, cb, 3] = np.asarray(gn_w, np.float32)[sl]
        vecs[:, cb, 4] = np.asarray(gn_b, np.float32)[sl]
    vecs = np.ascontiguousarray(vecs.reshape(P, NCB * 5))

    p_idx = np.arange(P)
    indr = np.zeros((P, NCB * NG), np.float32)
    indb = np.zeros((NG, C), np.float32)
    for cb in range(NCB):
        g_glob = 8 * cb + p_idx // GS
        # tile 3's stats arrive as raw [sum, sumsq] (ACT accum path);
        # tiles 0-2 as per-channel [mean, mean^2+var]
        scale = 1.0 / GS if cb < NCB - 1 else 1.0 / (GS * HW)
        indr[p_idx, cb * NG + g_glob] = scale
        indb[g_glob, cb * P + p_idx] = 1.0

    shared = dict(gT=gT, wvoT=wvoT, vecs=vecs,
                  indr=indr, indb=indb)
    in_maps = []
    for core in range(8):
        b, half = core // 2, core % 2
        xr = xs[b] if half == 0 else np.ascontiguousarray(
            np.roll(xs[b], -IQ, axis=1))
        m = dict(shared)
        m["x"] = xr
        m["xh"] = xr.astype(ml_dtypes.bfloat16)
        m["xhT"] = np.ascontiguousarray(xr.T).astype(ml_dtypes.bfloat16)
        in_maps.append(m)
    return in_maps


def kernel(x, gn_w, gn_b, wq, bq, wk, bk, wv, bv, wo, bo):
    global LAST_EXEC_TIME_NS
    nc = _build_nc()
    in_maps = _host_inputs(x, gn_w, gn_b, wq, bq, wk, bk, wv, bv, wo, bo)

    trace = os.environ.get("BASS_PROBLEM_TRACE", "") == "1"
    if trace:
        _install_profile_hook()
    res = run_bass_kernel_spmd(nc, in_maps, core_ids=list(range(8)), trace=trace)
    LAST_EXEC_TIME_NS = res.exec_time_ns
    global LAST_RESULTS
    LAST_RESULTS = res

    B, H = 4, 64
    out = np.empty((B, C, HW), np.float32)
    for core in range(8):
        b, half = core // 2, core % 2
        out[b][:, half * IQ:(half + 1) * IQ] = res.results[core]["y"]
    return out.reshape(B, C, H, H)


def _install_profile_hook():
    """Dev-only: register the NTFF profile hook trn_boot couldn't install
    (antenv.axon_hooks is absent in this image) and stub the artifact
    upload (no egress)."""
    import sys
    import types
    try:
        from trn_agent_boot.trn_boot import _ntff_profile_via_ctypes
        import antenv
    except ImportError:
        return
    if "antenv.axon_hooks" in sys.modules:
        return
    hook = _ntff_profile_via_ctypes('/opt/axon/libaxon_pjrt.so')
    mod = types.ModuleType("antenv.axon_hooks")
    mod.get_axon_ntff_profile_hook = lambda: hook
    sys.modules["antenv.axon_hooks"] = mod
    antenv.axon_hooks = mod
    import concourse.bass_utils as bu
    bu.upload_artifacts = lambda tmpdir: tmpdir

